# revision 17
# baseline (speedup 1.0000x reference)
"""Trainium2 Bass kernel for nn_Net_14869176779172 (moe_routing).

Computes, for x[B=1024, D=4096, S=60], W[D, S], soma_w[D], soma_b[1]:
    d[b, j]  = sum_s x[b, j, s] * W[j, s]          (per-dendrite dot)
    r        = relu(d)
    act[b,j] = sigmoid(r)        for j < 1638      (first 40% of dendrites)
             = sqrt(r)           otherwise
    out[b]   = act[b, :] @ soma_w + soma_b         -> [B, 1]

Sharding: pure data-parallel over batch across 8 NeuronCores (128 batch
rows per core); W / soma_w / soma_b replicated.

Per-core layout (v4): DENDRITES on the 128 SBUF partitions (x is
host-transposed + cast to fp16).  Each group g of 4 dendrite-chunks
(512 dendrites) is DMA'd as one [128, 4*128*60] fp16 tile.  The W
multiply is one in-place VectorE tensor_tensor (2x fp16 mode) against a
stride-0-broadcast W access pattern (W is fully resident: 60 fp16 per
dendrite per chunk).  The reduction over s is done on the otherwise-idle
TensorEngine: 60 accumulating matmuls per group with a reused fp16
identity stationary, each streaming the s-th slice (strided AP, N=512)
into a PSUM bank: z[d, (j,b)] += y[d, (j,b), s].  ScalarE applies
relu then sigmoid/sqrt out of PSUM.  The soma dot (a partition-axis
reduction) is 4 more tiny matmuls per group with soma_w slices as the
stationary, accumulated in a second PSUM region across all 32 chunks.
"""

import numpy as np

import concourse.bacc as bacc
import concourse.bass as bass
import concourse.tile as tile
from concourse import mybir
from concourse.bass_utils import run_bass_kernel_spmd

# Problem constants (hardcoded per harness contract).
B_TOTAL = 1024
N_CORES = 8
B = B_TOTAL // N_CORES  # 128 batch rows per core
D = 4096
S = 60
CUT = int(D * 0.4)  # 1638: first CUT dendrites use sigmoid, rest sqrt

P = 128  # SBUF partitions
N_CHUNKS = D // P  # 32 chunks of 128 dendrites
G_SUB = 2  # chunks per group
N_GROUPS = N_CHUNKS // G_SUB  # 8
GROUP_F = G_SUB * B * S  # 30720 fp16 elements per partition per group

FP32 = mybir.dt.float32
FP16 = mybir.dt.float16
FP8 = mybir.dt.float8e4
N_FP8_GROUPS = 6  # groups 0-5 = chunks 0-11 (sigmoid region) in fp8


def _build_program():
    nc = bacc.Bacc(
        "TRN2",
        target_bir_lowering=False,
        debug=False,
        enable_asserts=False,
        num_devices=N_CORES,
    )

    # Host-prearranged inputs (see kernel() below for the exact layouts).
    x8_d = nc.dram_tensor("xT8", [N_FP8_GROUPS, P, GROUP_F], FP8, kind="ExternalInput")
    x_d = nc.dram_tensor("xT16", [N_GROUPS - N_FP8_GROUPS, P, GROUP_F], FP16, kind="ExternalInput")
    w_d = nc.dram_tensor("WT", [P, N_CHUNKS * S], FP16, kind="ExternalInput")
    # soma_w columns per chunk; the CUT-straddling chunk (12) gets TWO
    # masked columns (sigmoid rows / sqrt rows) at indices 32 and 33.
    sw_d = nc.dram_tensor("swT", [P, N_CHUNKS + 2], FP16, kind="ExternalInput")
    id_d = nc.dram_tensor("ident", [P, P], FP16, kind="ExternalInput")
    sb_d = nc.dram_tensor("soma_b", [1], FP32, kind="ExternalInput")
    out_d = nc.dram_tensor("out", [B, 1], FP32, kind="ExternalOutput")

    with tile.TileContext(nc) as tc:
        with (
            tc.tile_pool(name="singles", bufs=1) as singles,
            tc.tile_pool(name="xpool", bufs=4) as xpool,
            tc.tile_pool(name="x8pool", bufs=2) as x8pool,
            tc.tile_pool(name="rpool", bufs=2) as rpool,
            tc.tile_pool(name="apool", bufs=3) as apool,
            tc.tile_pool(name="zfpool", bufs=2) as zfpool,
            tc.tile_pool(name="zpsum", bufs=6, space="PSUM") as zpsum,
            tc.tile_pool(name="spsum", bufs=1, space="PSUM") as spsum,
        ):
            # ---- small resident inputs (scalar-engine HWDGE ring, so the
            # first big x DMA on the sync ring is not delayed) ----
            w_t = singles.tile([P, N_CHUNKS * S], FP16)
            nc.sync.dma_start(out=w_t, in_=w_d.ap())
            ident_t = singles.tile([P, P], FP16)
            nc.scalar.dma_start(out=ident_t, in_=id_d.ap())
            sw_t = singles.tile([P, N_CHUNKS + 2], FP16)
            nc.scalar.dma_start(out=sw_t, in_=sw_d.ap())
            sb_t = singles.tile([1, 1], FP32)
            sb_ap = sb_d.ap()
            nc.scalar.dma_start(
                out=sb_t,
                in_=bass.AP(tensor=sb_ap.tensor, offset=sb_ap.offset, ap=[[0, 1], [1, 1]]),
            )

            soma_ps = spsum.tile([1, B], FP32)

            # Software-pipelined emission: group g's DMA/mul/bursts are
            # emitted first; group g-1's folds (DVE), relu/act (ScalarE)
            # follow the NEXT mul in each engine's FIFO, and group g-2's
            # soma matmuls slot between burst blocks on the PE -- so no
            # engine queue ever head-of-line blocks on another engine.
            state = {}  # g -> (zbanks, zf, r_t, a_t, a12)

            def emit_load_mul(g):
                xg = xpool.tile([P, GROUP_F], FP16)
                if g < N_FP8_GROUPS:
                    x8 = x8pool.tile([P, GROUP_F], FP8)
                    nc.sync.dma_start(out=x8, in_=x8_d.ap()[g])
                    # upcast fp8 -> fp16 on the (otherwise idle) scalar /
                    # gpsimd engines, alternating
                    if g % 2 == 0:
                        nc.scalar.copy(xg, x8)
                    else:
                        nc.gpsimd.tensor_copy(xg, x8)
                else:
                    nc.sync.dma_start(out=xg, in_=x_d.ap()[g - N_FP8_GROUPS])
                xr = xg.rearrange("p (j b s) -> p j b s", j=G_SUB, s=S)
                ws = w_t[:, g * G_SUB * S : (g + 1) * G_SUB * S]
                wb = bass.AP(
                    tensor=ws.tensor,
                    offset=ws.offset,
                    ap=[ws.ap[0], [S, G_SUB], [0, B], [1, S]],
                )
                nc.vector.tensor_mul(xr, xr, wb)
                return xr

            def emit_bursts(g, xr):
                zbanks = []
                for j in range(G_SUB):
                    for h in range(2):
                        z8 = zpsum.tile([P, 64 * 8], FP32)
                        z8v = z8.rearrange("p (b s) -> p b s", s=8)
                        for k in range(8):
                            s0 = 8 * k
                            sw_ = min(8, S - s0)
                            nc.tensor.matmul(
                                z8v[:, :, 0:sw_],
                                ident_t,
                                xr[:, j : j + 1, h * 64 : (h + 1) * 64, s0 : s0 + sw_],
                                start=(k == 0),
                                stop=(k == 7),
                            )
                        zbanks.append(z8v)
                state[g] = [zbanks]

            def emit_folds_act(g):
                zbanks, = state[g]
                zf = zfpool.tile([P, G_SUB * B], FP32)
                for i, z8v in enumerate(zbanks):
                    nc.vector.tensor_reduce(
                        out=zf[:, i * 64 : (i + 1) * 64],
                        in_=z8v,
                        axis=mybir.AxisListType.X,
                        op=mybir.AluOpType.add,
                    )
                r_t = rpool.tile([P, G_SUB * B], FP16)
                nc.scalar.activation(r_t, zf, mybir.ActivationFunctionType.Relu)
                a_t = apool.tile([P, G_SUB * B], FP16)
                a12 = None
                for j in range(G_SUB):
                    c = g * G_SUB + j
                    d0 = c * P
                    cols = slice(j * B, (j + 1) * B)
                    if d0 + P <= CUT:
                        nc.scalar.activation(
                            a_t[:, cols], r_t[:, cols],
                            mybir.ActivationFunctionType.Sigmoid,
                        )
                    elif d0 >= CUT:
                        nc.scalar.activation(
                            a_t[:, cols], r_t[:, cols],
                            mybir.ActivationFunctionType.Sqrt,
                        )
                    else:
                        nc.scalar.activation(
                            a_t[:, cols], r_t[:, cols],
                            mybir.ActivationFunctionType.Sigmoid,
                        )
                        a12 = apool.tile([P, B], FP16)
                        nc.scalar.activation(
                            a12, r_t[:, cols],
                            mybir.ActivationFunctionType.Sqrt,
                        )
                state[g] = [a_t, a12]

            def emit_soma(g):
                a_t, a12 = state.pop(g)
                for j in range(G_SUB):
                    c = g * G_SUB + j
                    d0 = c * P
                    straddle = d0 < CUT < d0 + P
                    sw_col = N_CHUNKS if straddle else c
                    nc.tensor.matmul(
                        soma_ps,
                        sw_t[:, sw_col : sw_col + 1],
                        a_t[:, j * B : (j + 1) * B],
                        start=(c == 0),
                        stop=(c == N_CHUNKS - 1) and not straddle,
                        skip_group_check=True,
                    )
                    if straddle:
                        nc.tensor.matmul(
                            soma_ps,
                            sw_t[:, N_CHUNKS + 1 : N_CHUNKS + 2],
                            a12,
                            start=False,
                            stop=(c == N_CHUNKS - 1),
                            skip_group_check=True,
                        )

            for g in range(N_GROUPS + 2):
                xr = emit_load_mul(g) if g < N_GROUPS else None
                if 1 <= g <= N_GROUPS:
                    emit_folds_act(g - 1)
                if g >= 2:
                    emit_soma(g - 2)
                if xr is not None:
                    emit_bursts(g, xr)

            # ---- finish: add soma_b, write out ----
            out_sb = singles.tile([1, B], FP32)
            nc.vector.tensor_scalar_add(out=out_sb, in0=soma_ps, scalar1=sb_t)
            out_ap = out_d.ap()
            nc.sync.dma_start(
                out=bass.AP(tensor=out_ap.tensor, offset=0, ap=[[0, 1], [1, B]]),
                in_=out_sb,
            )

    nc.compile()
    return nc


_NC_CACHE = None


def _get_program():
    global _NC_CACHE
    if _NC_CACHE is None:
        _NC_CACHE = _build_program()
    return _NC_CACHE


def kernel(x, W, soma_w, soma_b, _trace=False):
    nc = _get_program()
    x = np.asarray(x, dtype=np.float32)
    W = np.asarray(W, dtype=np.float32)
    soma_w = np.asarray(soma_w, dtype=np.float32)
    soma_b = np.asarray(soma_b, dtype=np.float32)

    # W layout: [p, c*60+s] = W[c*128+p, s]
    wT = np.ascontiguousarray(
        W.reshape(N_CHUNKS, P, S).transpose(1, 0, 2).reshape(P, N_CHUNKS * S)
    ).astype(np.float16)
    # soma_w layout: [p, c] = soma_w[c*128+p]; plus two masked columns for
    # the CUT-straddling chunk (sigmoid rows / sqrt rows).
    swT = np.zeros((P, N_CHUNKS + 2), dtype=np.float16)
    swT[:, :N_CHUNKS] = soma_w.reshape(N_CHUNKS, P).T.astype(np.float16)
    c12 = CUT // P  # 12
    p_cut = CUT - c12 * P  # 102
    swT[:p_cut, N_CHUNKS] = swT[:p_cut, c12]
    swT[p_cut:, N_CHUNKS + 1] = swT[p_cut:, c12]
    swT[:, c12] = 0  # unused for the straddling chunk
    ident = np.eye(P, dtype=np.float16)

    fp8_np = mybir.dt.np(FP8)
    in_maps = []
    for i in range(N_CORES):
        xc = x[i * B : (i + 1) * B]  # [128, 4096, 60]
        # xT[g, p, j, b, s] = x[b, (G_SUB*g+j)*128+p, s]
        xT = (
            xc.reshape(B, N_GROUPS, G_SUB, P, S)
            .transpose(1, 3, 2, 0, 4)
            .reshape(N_GROUPS, P, GROUP_F)
        )
        xT8 = np.ascontiguousarray(xT[:N_FP8_GROUPS]).astype(fp8_np)
        xT16 = np.ascontiguousarray(xT[N_FP8_GROUPS:]).astype(np.float16)
        in_maps.append(
            {
                "xT8": xT8,
                "xT16": xT16,
                "WT": wT,
                "swT": swT,
                "ident": ident,
                "soma_b": soma_b,
            }
        )
    res = run_bass_kernel_spmd(
        nc, in_maps, core_ids=list(range(N_CORES)), trace=_trace
    )
    out = np.concatenate([r["out"] for r in res.results], axis=0)
    if _trace:
        kernel.last_results = res
    return out.astype(np.float32)


# revision 19
# speedup vs baseline: 1.6427x; 1.6427x over previous
"""Trainium2 Bass kernel for nn_Net_14869176779172 (moe_routing).

Computes, for x[B=1024, D=4096, S=60], W[D, S], soma_w[D], soma_b[1]:
    d[b, j]  = sum_s x[b, j, s] * W[j, s]          (per-dendrite dot)
    r        = relu(d)
    act[b,j] = sigmoid(r)        for j < 1638      (first 40% of dendrites)
             = sqrt(r)           otherwise
    out[b]   = act[b, :] @ soma_w + soma_b         -> [B, 1]

Sharding: pure data-parallel over batch across 8 NeuronCores (128 batch
rows per core); W / soma_w / soma_b replicated.

Per-core layout (v4): DENDRITES on the 128 SBUF partitions (x is
host-transposed + cast to fp16).  Each group g of 4 dendrite-chunks
(512 dendrites) is DMA'd as one [128, 4*128*60] fp16 tile.  The W
multiply is one in-place VectorE tensor_tensor (2x fp16 mode) against a
stride-0-broadcast W access pattern (W is fully resident: 60 fp16 per
dendrite per chunk).  The reduction over s is done on the otherwise-idle
TensorEngine: 60 accumulating matmuls per group with a reused fp16
identity stationary, each streaming the s-th slice (strided AP, N=512)
into a PSUM bank: z[d, (j,b)] += y[d, (j,b), s].  ScalarE applies
relu then sigmoid/sqrt out of PSUM.  The soma dot (a partition-axis
reduction) is 4 more tiny matmuls per group with soma_w slices as the
stationary, accumulated in a second PSUM region across all 32 chunks.
"""

import numpy as np

import concourse.bacc as bacc
import concourse.bass as bass
import concourse.tile as tile
from concourse import mybir
from concourse.bass_utils import run_bass_kernel_spmd

# Problem constants (hardcoded per harness contract).
B_TOTAL = 1024
N_CORES = 8
B = B_TOTAL // N_CORES  # 128 batch rows per core
D = 4096
S = 60
CUT = int(D * 0.4)  # 1638: first CUT dendrites use sigmoid, rest sqrt

P = 128  # SBUF partitions
N_CHUNKS = D // P  # 32 chunks of 128 dendrites
G_SUB = 2  # chunks per group
N_GROUPS = N_CHUNKS // G_SUB  # 8
GROUP_F = G_SUB * B * S  # 30720 fp16 elements per partition per group

FP32 = mybir.dt.float32
FP16 = mybir.dt.float16
FP8 = mybir.dt.float8e4
N_FP8_GROUPS = 6  # groups 0-5 = chunks 0-11 (sigmoid region) in fp8


def _build_program():
    nc = bacc.Bacc(
        "TRN2",
        target_bir_lowering=False,
        debug=False,
        enable_asserts=False,
        num_devices=N_CORES,
    )

    # Host-prearranged inputs (see kernel() below for the exact layouts).
    x8_d = nc.dram_tensor("xT8", [N_FP8_GROUPS, P, GROUP_F], FP8, kind="ExternalInput")
    x_d = nc.dram_tensor("xT16", [N_GROUPS - N_FP8_GROUPS, P, GROUP_F], FP16, kind="ExternalInput")
    w_d = nc.dram_tensor("WT", [P, N_CHUNKS * S], FP16, kind="ExternalInput")
    # soma_w columns per chunk; the CUT-straddling chunk (12) gets TWO
    # masked columns (sigmoid rows / sqrt rows) at indices 32 and 33.
    sw_d = nc.dram_tensor("swT", [P, N_CHUNKS + 2], FP16, kind="ExternalInput")
    id_d = nc.dram_tensor("ident", [P, P], FP16, kind="ExternalInput")
    sb_d = nc.dram_tensor("soma_b", [1], FP32, kind="ExternalInput")
    out_d = nc.dram_tensor("out", [B, 1], FP32, kind="ExternalOutput")

    with tile.TileContext(nc) as tc:
        with (
            tc.tile_pool(name="singles", bufs=1) as singles,
            tc.tile_pool(name="xpool", bufs=4) as xpool,
            tc.tile_pool(name="x8pool", bufs=2) as x8pool,
            tc.tile_pool(name="rpool", bufs=2) as rpool,
            tc.tile_pool(name="zfpool", bufs=2) as zfpool,
            tc.tile_pool(name="apool", bufs=3) as apool,
            tc.tile_pool(name="zpsum", bufs=6, space="PSUM") as zpsum,
            tc.tile_pool(name="spsum", bufs=1, space="PSUM") as spsum,
        ):
            # ---- small resident inputs (scalar-engine HWDGE ring, so the
            # first big x DMA on the sync ring is not delayed) ----
            w_t = singles.tile([P, N_CHUNKS * S], FP16)
            nc.sync.dma_start(out=w_t, in_=w_d.ap())
            ident_t = singles.tile([P, P], FP16)
            nc.scalar.dma_start(out=ident_t, in_=id_d.ap())
            sw_t = singles.tile([P, N_CHUNKS + 2], FP16)
            nc.scalar.dma_start(out=sw_t, in_=sw_d.ap())
            sb_t = singles.tile([1, 1], FP32)
            sb_ap = sb_d.ap()
            nc.scalar.dma_start(
                out=sb_t,
                in_=bass.AP(tensor=sb_ap.tensor, offset=sb_ap.offset, ap=[[0, 1], [1, 1]]),
            )

            soma_ps = spsum.tile([1, B], FP32)

            # Software-pipelined emission: group g's DMA/mul/bursts are
            # emitted first; group g-1's folds (DVE), relu/act (ScalarE)
            # follow the NEXT mul in each engine's FIFO, and group g-2's
            # soma matmuls slot between burst blocks on the PE -- so no
            # engine queue ever head-of-line blocks on another engine.
            state = {}  # g -> (zbanks, zf, r_t, a_t, a12)

            def emit_load_mul(g):
                xg = xpool.tile([P, GROUP_F], FP16)
                if g < N_FP8_GROUPS:
                    x8 = x8pool.tile([P, GROUP_F], FP8)
                    nc.sync.dma_start(out=x8, in_=x8_d.ap()[g])
                    # upcast fp8 -> fp16 on the (otherwise idle) scalar
                    # engine.  (GpSimd CAST measured 54us AND stretches
                    # concurrent DVE ops ~6x -- do not use it here.)
                    nc.scalar.copy(xg, x8)
                else:
                    nc.sync.dma_start(out=xg, in_=x_d.ap()[g - N_FP8_GROUPS])
                xr = xg.rearrange("p (j b s) -> p j b s", j=G_SUB, s=S)
                ws = w_t[:, g * G_SUB * S : (g + 1) * G_SUB * S]
                wb = bass.AP(
                    tensor=ws.tensor,
                    offset=ws.offset,
                    ap=[ws.ap[0], [S, G_SUB], [0, B], [1, S]],
                )
                nc.vector.tensor_mul(xr, xr, wb)
                return xr

            def emit_bursts(g, xr):
                zbanks = []
                for j in range(G_SUB):
                    for h in range(2):
                        z8 = zpsum.tile([P, 64 * 8], FP32)
                        z8v = z8.rearrange("p (b s) -> p b s", s=8)
                        for k in range(8):
                            s0 = 8 * k
                            sw_ = min(8, S - s0)
                            nc.tensor.matmul(
                                z8v[:, :, 0:sw_],
                                ident_t,
                                xr[:, j : j + 1, h * 64 : (h + 1) * 64, s0 : s0 + sw_],
                                start=(k == 0),
                                stop=(k == 7),
                            )
                        zbanks.append(z8v)
                state[g] = [zbanks]

            def emit_act(g):
                zbanks, = state[g]
                zf = zfpool.tile([P, G_SUB * B], FP32)
                for i, z8v in enumerate(zbanks):
                    nc.vector.tensor_reduce(
                        out=zf[:, i * 64 : (i + 1) * 64],
                        in_=z8v,
                        axis=mybir.AxisListType.X,
                        op=mybir.AluOpType.add,
                    )
                r_t = rpool.tile([P, G_SUB * B], FP16)
                nc.scalar.activation(r_t, zf, mybir.ActivationFunctionType.Relu)
                a_t = apool.tile([P, G_SUB * B], FP16)
                a12 = None
                for j in range(G_SUB):
                    c = g * G_SUB + j
                    d0 = c * P
                    cols = slice(j * B, (j + 1) * B)
                    if d0 + P <= CUT:
                        nc.scalar.activation(
                            a_t[:, cols], r_t[:, cols],
                            mybir.ActivationFunctionType.Sigmoid,
                        )
                    elif d0 >= CUT:
                        nc.scalar.activation(
                            a_t[:, cols], r_t[:, cols],
                            mybir.ActivationFunctionType.Sqrt,
                        )
                    else:
                        nc.scalar.activation(
                            a_t[:, cols], r_t[:, cols],
                            mybir.ActivationFunctionType.Sigmoid,
                        )
                        a12 = apool.tile([P, B], FP16)
                        nc.scalar.activation(
                            a12, r_t[:, cols],
                            mybir.ActivationFunctionType.Sqrt,
                        )
                state[g] = [a_t, a12]

            def emit_soma(g):
                a_t, a12 = state.pop(g)
                for j in range(G_SUB):
                    c = g * G_SUB + j
                    d0 = c * P
                    straddle = d0 < CUT < d0 + P
                    sw_col = N_CHUNKS if straddle else c
                    nc.tensor.matmul(
                        soma_ps,
                        sw_t[:, sw_col : sw_col + 1],
                        a_t[:, j * B : (j + 1) * B],
                        start=(c == 0),
                        stop=(c == N_CHUNKS - 1) and not straddle,
                        skip_group_check=True,
                    )
                    if straddle:
                        nc.tensor.matmul(
                            soma_ps,
                            sw_t[:, N_CHUNKS + 1 : N_CHUNKS + 2],
                            a12,
                            start=False,
                            stop=(c == N_CHUNKS - 1),
                            skip_group_check=True,
                        )

            # Interleave fp8 and fp16 groups so the scalar upcasts and
            # the larger fp16 DMAs spread evenly across the kernel.
            order = []
            for i in range(N_FP8_GROUPS):
                order += [i, N_FP8_GROUPS + i]
            order += list(range(2 * N_FP8_GROUPS, N_GROUPS))
            for i in range(N_GROUPS + 2):
                xr = emit_load_mul(order[i]) if i < N_GROUPS else None
                if 1 <= i <= N_GROUPS:
                    emit_act(order[i - 1])
                if i >= 2:
                    emit_soma(order[i - 2])
                if xr is not None:
                    emit_bursts(order[i], xr)

            # ---- finish: add soma_b, write out ----
            out_sb = singles.tile([1, B], FP32)
            nc.vector.tensor_scalar_add(out=out_sb, in0=soma_ps, scalar1=sb_t)
            out_ap = out_d.ap()
            nc.sync.dma_start(
                out=bass.AP(tensor=out_ap.tensor, offset=0, ap=[[0, 1], [1, B]]),
                in_=out_sb,
            )

    nc.compile()
    return nc


_NC_CACHE = None


def _get_program():
    global _NC_CACHE
    if _NC_CACHE is None:
        _NC_CACHE = _build_program()
    return _NC_CACHE


def kernel(x, W, soma_w, soma_b, _trace=False):
    nc = _get_program()
    x = np.asarray(x, dtype=np.float32)
    W = np.asarray(W, dtype=np.float32)
    soma_w = np.asarray(soma_w, dtype=np.float32)
    soma_b = np.asarray(soma_b, dtype=np.float32)

    # W layout: [p, c*60+s] = W[c*128+p, s]
    wT = np.ascontiguousarray(
        W.reshape(N_CHUNKS, P, S).transpose(1, 0, 2).reshape(P, N_CHUNKS * S)
    ).astype(np.float16)
    # soma_w layout: [p, c] = soma_w[c*128+p]; plus two masked columns for
    # the CUT-straddling chunk (sigmoid rows / sqrt rows).
    swT = np.zeros((P, N_CHUNKS + 2), dtype=np.float16)
    swT[:, :N_CHUNKS] = soma_w.reshape(N_CHUNKS, P).T.astype(np.float16)
    c12 = CUT // P  # 12
    p_cut = CUT - c12 * P  # 102
    swT[:p_cut, N_CHUNKS] = swT[:p_cut, c12]
    swT[p_cut:, N_CHUNKS + 1] = swT[p_cut:, c12]
    swT[:, c12] = 0  # unused for the straddling chunk
    ident = np.eye(P, dtype=np.float16)

    fp8_np = mybir.dt.np(FP8)
    in_maps = []
    for i in range(N_CORES):
        xc = x[i * B : (i + 1) * B]  # [128, 4096, 60]
        # xT[g, p, j, b, s] = x[b, (G_SUB*g+j)*128+p, s]
        xT = (
            xc.reshape(B, N_GROUPS, G_SUB, P, S)
            .transpose(1, 3, 2, 0, 4)
            .reshape(N_GROUPS, P, GROUP_F)
        )
        xT8 = np.ascontiguousarray(xT[:N_FP8_GROUPS]).astype(fp8_np)
        xT16 = np.ascontiguousarray(xT[N_FP8_GROUPS:]).astype(np.float16)
        in_maps.append(
            {
                "xT8": xT8,
                "xT16": xT16,
                "WT": wT,
                "swT": swT,
                "ident": ident,
                "soma_b": soma_b,
            }
        )
    res = run_bass_kernel_spmd(
        nc, in_maps, core_ids=list(range(N_CORES)), trace=_trace
    )
    out = np.concatenate([r["out"] for r in res.results], axis=0)
    if _trace:
        kernel.last_results = res
    return out.astype(np.float32)
